# revision 19
# baseline (speedup 1.0000x reference)
"""Conditional VQ (codebook) Trainium2 kernel.

Problem: x (n=32768, g=4, d=128) f32, code_book (g=4, k=512, d=128) f32.
  dist[t,g,k] = ||x[t,g]||^2 + ||cb[g,k]||^2 - 2 x[t,g].cb[g,k]
  index = argmin_k dist                      (n, g, 1) int32
  one_hot = onehot(index)                    (n, g, 512) f32
  x_hat = cb[g, index]                       (n, g, 128) f32

argmin_k dist == argmax_k score, score[t,g,k] = x.cb - 0.5||cb||^2  (x2 drops).

Sharding: data-parallel over n across 8 cores (4096 tokens/core);
codebook replicated.

Per-core kernel (per 128-token block, per group g):
  PE:   transpose x-tile -> xT; bias matmul (K=1 ones x -0.5cb2 row,
        start=True) then cross matmul (xT.T @ cbT) accumulated in PSUM.
  ACT:  copy scores PSUM->SBUF.
  DVE:  max8 + find_index8 -> argmax index per row.
  GPS:  one_hot = is_equal(scores, rowmax); flat offsets for gather.
  DMA:  indirect gather code_book rows -> x_hat; stores.
"""

import os
import sys

import numpy as np

for _p in ("/opt/trn_rl_repo",):
    if _p not in sys.path:
        sys.path.insert(0, _p)

import concourse.bacc as bacc
import concourse.bass as bass
import concourse.tile as tile
from concourse import mybir
from concourse.bass_utils import run_bass_kernel_spmd

F32 = mybir.dt.float32
F32R = mybir.dt.float32r
U32 = mybir.dt.uint32
I32 = mybir.dt.int32

N_CORES = 8
N_TOK = 32768
G = 4
D = 128
K = 512
# per-core token count; overridable only for small simulator runs
N_LOC = int(os.environ.get("VQ_NLOC", N_TOK // N_CORES))
BLK = 128                         # tokens per block
NBLK = N_LOC // BLK               # 32 blocks

# matmul compute dtype: "f32r" (fast fp32 mode, 1 cyc/row at N>=256) or "f32"
MM_DTYPE = os.environ.get("VQ_MM_DTYPE", "f32")


def _mm_cast(ap):
    if MM_DTYPE == "f32r":
        return ap.bitcast(F32R)
    return ap


def build_nc():
    nc = bacc.Bacc(
        "TRN2",
        target_bir_lowering=False,
        debug=False,
        num_devices=N_CORES,
    )

    x_in = nc.declare_dram_parameter("x", [N_LOC, G, D], F32, isOutput=False)
    cb_in = nc.declare_dram_parameter("code_book", [G, K, D], F32, isOutput=False)
    eye_in = nc.declare_dram_parameter("eye128", [128, 128], F32, isOutput=False)

    xhat_out = nc.declare_dram_parameter("x_hat", [N_LOC, G, D], F32, isOutput=True)
    oh_out = nc.declare_dram_parameter("one_hot", [N_LOC, G, K], F32, isOutput=True)
    idx_out = nc.declare_dram_parameter("index", [N_LOC, G, 1], I32, isOutput=True)

    with tile.TileContext(nc) as tc:
        with (
            tc.tile_pool(name="const", bufs=1) as const_pool,
            tc.tile_pool(name="cbload", bufs=2) as cbload_pool,
            tc.tile_pool(name="xload", bufs=3) as xload_pool,
            tc.tile_pool(name="xt_ps", bufs=2, space="PSUM") as xt_ps_pool,
            tc.tile_pool(name="xt_sb", bufs=3) as xt_sb_pool,
            tc.tile_pool(name="sc_ps", bufs=3, space="PSUM") as sc_ps_pool,
            tc.tile_pool(name="sc_sb", bufs=3) as sc_sb_pool,
            tc.tile_pool(name="mx", bufs=4) as mx_pool,
            tc.tile_pool(name="oh", bufs=2) as oh_pool,
            tc.tile_pool(name="xh", bufs=2) as xh_pool,
            tc.tile_pool(name="small", bufs=2) as small_pool,
        ):
            # ---- constants ----
            eye_sb = const_pool.tile([128, 128], F32, tag="eye")
            nc.sync.dma_start(eye_sb[:, :], eye_in[:, :])

            # Constants produced on ACT so matmuls that consume them share a
            # single ACT vector-clock wait (walrus LDW allows few sem waits).
            ones_col = const_pool.tile([128, 1], F32, tag="ones_col")
            nc.scalar.activation(ones_col[:, :], eye_sb[:, 0:1],
                                 mybir.ActivationFunctionType.Copy,
                                 bias=1.0, scale=0.0)
            ones_row = const_pool.tile([1, 128], F32, tag="ones_row")
            nc.scalar.activation(ones_row[:, :], eye_sb[0:1, :],
                                 mybir.ActivationFunctionType.Copy,
                                 bias=1.0, scale=0.0)

            # Dummy PE transpose touching only eye: advances PE's vector
            # clock past the eye DMA so every later matmul carries at most
            # one embedded sem wait (walrus LDW limit).
            ps0 = xt_ps_pool.tile([128, 128], F32, tag="xt")
            nc.tensor.transpose(ps0[:, :], eye_sb[:, :], eye_sb[:, :])
            warm_sb = xt_sb_pool.tile([128, 128], F32, tag="warm")
            nc.scalar.copy(warm_sb[:, :], ps0[:, :])

            # ---- codebook: transposed copy cbT[g] = cb[g].T  [128d x 512k] ----
            cbt = const_pool.tile([128, G * K], F32, tag="cbt")  # 1 MiB
            for g in range(G):
                for s in range(K // 128):
                    ld = cbload_pool.tile([128, 128], F32)
                    nc.sync.dma_start(ld[:, :], cb_in[g, 128 * s:128 * (s + 1), :])
                    ps = xt_ps_pool.tile([128, 128], F32, tag="xt")
                    nc.tensor.transpose(ps[:, :], ld[:, :], eye_sb[:, :])
                    nc.scalar.copy(cbt[:, K * g + 128 * s:K * g + 128 * (s + 1)],
                                   ps[:, :])

            # ---- -0.5 * ||cb||^2 rows:  nhcb2[0, g*K + k] ----
            nhcb2 = const_pool.tile([1, G * K], F32, tag="nhcb2")
            for g in range(G):
                sq = sc_sb_pool.tile([128, K], F32, tag="sq")
                nc.scalar.square(sq[:, :], cbt[:, K * g:K * (g + 1)])
                ps2 = sc_ps_pool.tile([1, K], F32, tag="sc")
                nc.tensor.matmul(ps2[:, :], ones_col[:, :], sq[:, :],
                                 start=True, stop=True)
                nc.scalar.mul(nhcb2[:, K * g:K * (g + 1)], ps2[:, :], -0.5)

            # ---- main loop over 128-token blocks ----
            for b in range(NBLK):
                r0 = BLK * b
                xt_full = xload_pool.tile([128, G, D], F32, tag="xt_full")
                nc.sync.dma_start(xt_full[:, :, :], x_in[r0:r0 + BLK, :, :])

                oh_sb = oh_pool.tile([128, G, K], F32, tag="oh")
                off_sb = small_pool.tile([128, G], U32, tag="off")
                idx_sb = small_pool.tile([128, G], I32, tag="idx")

                for g in range(G):
                    # xT = x[:, g, :].T via PE
                    xt_ps = xt_ps_pool.tile([128, 128], F32, tag="xt")
                    nc.tensor.transpose(xt_ps[:, :], xt_full[:, g, :],
                                        eye_sb[:, :])
                    xt_sb = xt_sb_pool.tile([128, 128], F32)
                    nc.scalar.copy(xt_sb[:, :], xt_ps[:, :])

                    # scores = -0.5||cb||^2 (+) xT.T @ cbT   (PSUM accumulate)
                    sc_ps = sc_ps_pool.tile([128, K], F32, tag="sc")
                    nc.tensor.matmul(sc_ps[:, :],
                                     _mm_cast(ones_row[:, :]),
                                     _mm_cast(nhcb2[:, K * g:K * (g + 1)]),
                                     start=True, stop=False)
                    nc.tensor.matmul(sc_ps[:, :],
                                     _mm_cast(xt_sb[:, :]),
                                     _mm_cast(cbt[:, K * g:K * (g + 1)]),
                                     start=False, stop=True)

                    sc_sb = sc_sb_pool.tile([128, K], F32, tag="sc_sb")
                    nc.scalar.copy(sc_sb[:, :], sc_ps[:, :])

                    # argmax along k
                    mx = mx_pool.tile([128, 8], F32, tag="mx")
                    nc.vector.max(mx[:, :], sc_sb[:, :])
                    mi = mx_pool.tile([128, 8], U32, tag="mi")
                    nc.vector.max_index(mi[:, :], mx[:, :], sc_sb[:, :])

                    # one_hot = (score == rowmax)
                    nc.gpsimd.tensor_scalar(oh_sb[:, g, :], sc_sb[:, :],
                                            mx[:, 0:1], None,
                                            mybir.AluOpType.is_equal)

                    # index out + flat gather offsets
                    nc.vector.tensor_copy(idx_sb[:, g:g + 1],
                                          mi[:, 0:1].bitcast(I32))
                    nc.vector.tensor_scalar(off_sb[:, g:g + 1], mi[:, 0:1],
                                            K * g, None, mybir.AluOpType.add)

                nc.sync.dma_start(oh_out[r0:r0 + BLK, :, :], oh_sb[:, :, :])
                nc.sync.dma_start(idx_out[r0:r0 + BLK, :, 0], idx_sb[:, :])

                # x_hat rows gathered from DRAM codebook (one row per
                # partition per group, canonical 2-D indirect gather form)
                xh_sb = xh_pool.tile([128, G, D], F32, tag="xh")
                cb_flat = cb_in[:, :, :].flatten_outer_dims()
                for g in range(G):
                    nc.gpsimd.indirect_dma_start(
                        xh_sb[:, g, :], None,
                        cb_flat,
                        bass.IndirectOffsetOnAxis(ap=off_sb[:, g:g + 1],
                                                  axis=0),
                    )
                nc.sync.dma_start(xhat_out[r0:r0 + BLK, :, :], xh_sb[:, :, :])

    nc.compile()
    return nc


_NC_CACHE = None
_JIT_CACHE = None


def _get_nc():
    global _NC_CACHE
    if _NC_CACHE is None:
        _NC_CACHE = build_nc()
    return _NC_CACHE


def _get_jitted():
    """Persistent jitted executable over the 8 cores (no donation, so the
    same device-resident buffers can be re-used across timed runs)."""
    global _JIT_CACHE
    if _JIT_CACHE is not None:
        return _JIT_CACHE

    import jax
    from jax.sharding import Mesh, PartitionSpec
    from jax.experimental.shard_map import shard_map
    from concourse import bass2jax

    nc = _get_nc()
    bass2jax.install_neuronx_cc_hook()

    partition_name = (nc.partition_id_tensor.name
                      if nc.partition_id_tensor else None)
    in_names: list[str] = []
    out_names: list[str] = []
    out_avals = []
    zero_shapes = []
    for alloc in nc.m.functions[0].allocations:
        if not isinstance(alloc, mybir.MemoryLocationSet):
            continue
        name = alloc.memorylocations[0].name
        if alloc.kind == "ExternalInput":
            if name != partition_name:
                in_names.append(name)
        elif alloc.kind == "ExternalOutput":
            out_names.append(name)
            shape = tuple(alloc.tensor_shape)
            dtype = mybir.dt.np(alloc.dtype)
            out_avals.append(jax.core.ShapedArray(shape, dtype))
            zero_shapes.append((shape, dtype))
    n_params = len(in_names)
    all_in_names = in_names + out_names
    if partition_name is not None:
        all_in_names = all_in_names + [partition_name]

    def _body(*args):
        operands = list(args)
        if partition_name is not None:
            operands.append(bass2jax.partition_id_tensor())
        outs = bass2jax._bass_exec_p.bind(
            *operands,
            out_avals=tuple(out_avals),
            in_names=tuple(all_in_names),
            out_names=tuple(out_names),
            lowering_input_output_aliases=(),
            sim_require_finite=True,
            sim_require_nnan=True,
            nc=nc,
        )
        return tuple(outs)

    devices = jax.devices()[:N_CORES]
    mesh = Mesh(np.asarray(devices), ("core",))
    n_all = n_params + len(out_names)
    donate = tuple(range(n_params, n_params + len(out_names)))
    sharded = jax.jit(
        shard_map(
            _body,
            mesh=mesh,
            in_specs=(PartitionSpec("core"),) * n_all,
            out_specs=(PartitionSpec("core"),) * len(out_names),
            check_rep=False,
        ),
        donate_argnums=donate,
        keep_unused=True,
    )
    _JIT_CACHE = (sharded, in_names, out_names, zero_shapes, mesh)
    return _JIT_CACHE


def _prep_inputs(x, code_book):
    x = np.ascontiguousarray(np.asarray(x, dtype=np.float32))
    code_book = np.ascontiguousarray(np.asarray(code_book, dtype=np.float32))
    assert x.shape == (N_TOK, G, D), x.shape
    assert code_book.shape == (G, K, D), code_book.shape
    eye = np.eye(128, dtype=np.float32)

    per_input = {
        "x": x,  # already (8*N_LOC, G, D) == concat of shards
        "code_book": np.concatenate([code_book] * N_CORES, axis=0),
        "eye128": np.concatenate([eye] * N_CORES, axis=0),
    }
    sharded_jit, in_names, out_names, zero_shapes, mesh = _get_jitted()
    args = [per_input[name] for name in in_names]
    zeros = [np.zeros((N_CORES * s[0], *s[1:]), dt) for s, dt in zero_shapes]
    return sharded_jit, args, zeros, out_names, mesh


def kernel(x, code_book):
    sharded_jit, args, zeros, out_names, _ = _prep_inputs(x, code_book)
    outs = sharded_jit(*args, *zeros)
    by_name = {n: np.asarray(o) for n, o in zip(out_names, outs)}
    return (by_name["x_hat"], by_name["one_hot"], by_name["index"])


def bench(x, code_book, iters=20):
    """Returns (outputs, per-iteration seconds) with device-resident args.

    Fresh donated zero output buffers are pre-created on device for every
    iteration so the timed loop measures only kernel executions.
    """
    import jax
    import jax.numpy as jnp
    import time
    from jax.sharding import NamedSharding, PartitionSpec

    sharded_jit, args, zeros, out_names, mesh = _prep_inputs(x, code_book)
    shard = NamedSharding(mesh, PartitionSpec("core"))
    dev_args = [jax.device_put(a, shard) for a in args]

    def dev_zeros():
        return [jnp.zeros(z.shape, z.dtype, device=shard) for z in zeros]

    outs = sharded_jit(*dev_args, *dev_zeros())  # compile + warm
    jax.block_until_ready(outs)

    all_zeros = [dev_zeros() for _ in range(iters)]
    jax.block_until_ready(all_zeros)

    t0 = time.perf_counter()
    for i in range(iters):
        outs = sharded_jit(*dev_args, *all_zeros[i])
    jax.block_until_ready(outs)
    t1 = time.perf_counter()

    by_name = {n: np.asarray(o) for n, o in zip(out_names, outs)}
    return (by_name["x_hat"], by_name["one_hot"], by_name["index"]), (t1 - t0) / iters
